# revision 6
# baseline (speedup 1.0000x reference)
"""Trainium2 Bass kernel for nn_PositionalEncoding_61151744360729.

out[b, s, n, :] = x[b, s, n, :] + ||x[b, s+1, n, :] - x[b, s, n, :]||_2
(with distance 0 at s = S-1).

Sharding: data-parallel on batch across 8 NeuronCores (64 batches/core).

Device layout: fp16 end-to-end, c-planar. Host repacks x to, per
(batch, seq-half) partition, [3 coord planes][SH+1 frames][26 nodes]
(nodes padded 25->26 so the one-frame shift is 52B = 4B-aligned and all
DVE tensor_tensor ops hit the 2x perf mode; fp16 I/O halves HBM traffic
vs fp32).

v4 structure (SDMA pool ~55us of transfer is the roofline):
- One batched 3D-AP input DMA per chunk, one output DMA per half-chunk
  piece (the per-DMA ~600-900ns SP sequencer cost made 76 small DMAs a
  near-critical 48us in the original version).
- Input DMAs staggered (chunk k+2 issued at chunk k's compute): the
  SDMA engines round-robin across queued DMAs, so queueing everything
  up front starves chunk 0's completion by ~8us.
- The engines execute their queues in order, so a back-end op whose
  producer hasn't finished stalls everything behind it. With back-end
  ops issued right after the next front (1-piece lag), the serial
  cycle add -> sub -> square -> sqrt (~11us/chunk) dominates: that was
  the old version's real limiter. Here sqrt lags its matmuls by 2
  pieces and add/output lag 5 pieces, so every op's inputs are long
  done when the engine reaches it and engines stream at busy-rate.
- PSUM pieces are [P, 1024] fp32 = 2 banks, 4 in flight, so the deep
  lag fits the 8 PSUM banks.
- Per-plane DVE subtracts (a fused 3-run-AP subtract measured ~10%
  slower than 3 single-run ops), squares on ACT (fused 3-plane op)
  except a DVE plane-2 square on some chunks to balance engines, plane
  sum as identity matmuls into PSUM, ACT sqrt per piece, one stride-0
  broadcast DVE add per piece.
- No GPSIMD tensor ops: the Pool engine shares its SBUF port with the
  DVE, and measured contention inflated concurrent DVE ops 25-100%.
"""

import sys
from contextlib import ExitStack

for _p in ("/opt/trn_rl_repo", "/root/.axon_site/_ro/trn_rl_repo"):
    if _p not in sys.path:
        sys.path.insert(0, _p)

import numpy as np

import concourse.bass as bass
import concourse.tile as tile
from concourse import bacc, mybir
from concourse.bass_utils import run_bass_kernel_spmd

B, S, N, C = 512, 1024, 25, 3
NCORES = 8
BC = B // NCORES           # 64 batches per core
H = 2                      # sequence halves -> 128 partitions
SH = S // H                # 512 frames per half
P = H * BC                 # 128 partitions
NP = 26                    # nodes padded to 26 (4B-aligned frame stride)
IN_PLANE = (SH + 1) * NP   # input elems per coord plane per partition
OUT_PLANE = SH * NP        # output elems per plane per partition
IN_FLAT = P * C * IN_PLANE
OUT_FLAT = P * C * OUT_PLANE
PSUM_W = 512               # one PSUM bank of fp32 per matmul window

F = 64                     # frames per chunk
K = SH // F                # 8 chunks
FI = (F + 1) * NP          # 1690 input elems per plane per chunk
FD = F * NP                # 1664 output elems per plane per chunk
HW = FD // 2               # 832: half-chunk elems per plane

# square of plane 2 per chunk: 'act' (fused into the 3-plane ACT op) or
# 'dve' (separate DVE multiply) -- tuned so DVE and ACT busy time match
SQ2 = ["act", "act", "act", "dve", "act", "act", "act", "dve"]

J = 2 * K                  # 16 half-chunk pieces
SQRT_LAG = 2               # sqrt of piece j issued at iteration j+SQRT_LAG
ADD_LAG = 5                # add+output of piece j at iteration j+ADD_LAG

_cache = {}


def _build():
    f16 = mybir.dt.float16
    f32 = mybir.dt.float32
    Af = mybir.ActivationFunctionType
    nc = bacc.Bacc(
        "TRN2", target_bir_lowering=False, debug=False, num_devices=NCORES
    )
    xin = nc.dram_tensor("xin", [IN_FLAT], f16, kind="ExternalInput")
    ident = nc.dram_tensor("ident", [P * P], f16, kind="ExternalInput")
    yout = nc.dram_tensor("yout", [OUT_FLAT], f16, kind="ExternalOutput")

    OFF = [k * F for k in range(K)]
    PF = 2                 # input prefetch depth in chunks

    with tile.TileContext(nc) as tc, ExitStack() as ctx:
        pconst = ctx.enter_context(tc.tile_pool(name="pconst", bufs=1))
        pin0 = ctx.enter_context(tc.tile_pool(name="pin0", bufs=3))
        pin = ctx.enter_context(tc.tile_pool(name="pin", bufs=6))
        pd = ctx.enter_context(tc.tile_pool(name="pd", bufs=3))
        ps = ctx.enter_context(tc.tile_pool(name="ps", bufs=5))
        po = ctx.enter_context(tc.tile_pool(name="po", bufs=3))
        ppsum = ctx.enter_context(
            tc.tile_pool(name="ppsum", bufs=4, space="PSUM")
        )

        # chunk 0 inputs first and per plane: they have the SDMA pool to
        # themselves while the SP sequencer works through later issues,
        # so the first subtract can start at ~3.5us
        x0 = []
        for c in range(C):
            t = pin0.tile([P, FI], f16)
            nc.sync.dma_start(
                t[:],
                bass.AP(xin, c * IN_PLANE, [[C * IN_PLANE, P], [1, FI]]),
            )
            x0.append(t)
        id_t = pconst.tile([P, P], f16)
        nc.sync.dma_start(id_t[:], bass.AP(ident, 0, [[P, P], [1, P]]))

        xk = [None] * K
        dk = [None] * K
        psum_p = [None] * J
        dist_p = [None] * J

        def issue_in(k):
            t = pin.tile([P, C * FI], f16)
            src = bass.AP(
                xin,
                OFF[k] * NP,
                [[C * IN_PLANE, P], [IN_PLANE, C], [1, FI]],
            )
            nc.sync.dma_start(t[:], src)
            xk[k] = t

        for k in range(1, 1 + PF):
            issue_in(k)

        def sub_sq(k):
            """per-plane subs + squares for chunk k (both pieces)"""
            d_t = pd.tile([P, C * FD], f16)
            dk[k] = d_t
            d3 = d_t[:].rearrange("p (c x) -> p c x", c=C)
            if k == 0:
                for c in range(C):
                    nc.vector.tensor_sub(
                        d3[:, c], x0[c][:, NP:NP + FD], x0[c][:, 0:FD]
                    )
            else:
                x3 = xk[k][:].rearrange("p (c f) -> p c f", c=C)
                for c in range(C):
                    nc.vector.tensor_sub(
                        d3[:, c], x3[:, c, NP:], x3[:, c, 0:FD]
                    )
            if SQ2[k] == "act":
                nc.scalar.activation(
                    d_t[:, 0:C * FD], d_t[:, 0:C * FD], Af.Square
                )
            else:
                nc.scalar.activation(
                    d_t[:, 0:2 * FD], d_t[:, 0:2 * FD], Af.Square
                )
                sq2 = d3[:, 2]
                nc.vector.tensor_mul(sq2, sq2, sq2)

        def mm(j):
            """plane-sum matmuls for piece j -> [P, 1024] 2-bank psum"""
            k, h = j // 2, j % 2
            d3 = dk[k][:].rearrange("p (c x) -> p c x", c=C)
            lo = h * HW
            ps_t = ppsum.tile([P, 1024], f32)
            psum_p[j] = ps_t
            for w0 in range(0, HW, PSUM_W):
                w1 = min(w0 + PSUM_W, HW)
                for c in range(C):
                    nc.tensor.matmul(
                        ps_t[:, w0:w1],
                        id_t[:],
                        d3[:, c, lo + w0:lo + w1],
                        start=(c == 0),
                        stop=(c == C - 1),
                    )

        def sqrt_p(j):
            s_t = ps.tile([P, HW], f16)
            dist_p[j] = s_t
            nc.scalar.activation(s_t[:], psum_p[j][:, 0:HW], Af.Sqrt)
            psum_p[j] = None

        def add_out(j):
            k, h = j // 2, j % 2
            lo, hi = h * HW, (h + 1) * HW
            s_t = dist_p[j]
            o_t = po.tile([P, C * HW], f16)
            o3 = o_t[:].rearrange("p (c x) -> p c x", c=C)
            if k == 0:
                for c in range(C):
                    nc.vector.tensor_add(
                        o3[:, c], x0[c][:, lo:hi], s_t[:]
                    )
            else:
                x3 = xk[k][:].rearrange("p (c f) -> p c f", c=C)
                sb = s_t[:].unsqueeze(1).broadcast_to([P, C, HW])
                nc.vector.tensor_add(o3, x3[:, :, lo:hi], sb)
            dst = bass.AP(
                yout,
                OFF[k] * NP + lo,
                [[C * OUT_PLANE, P], [OUT_PLANE, C], [1, HW]],
            )
            nc.sync.dma_start(dst, o3)
            dist_p[j] = None

        # deep-lag software pipeline over 16 half-chunk pieces
        for j in range(J + ADD_LAG):
            if j < J:
                k, h = j // 2, j % 2
                if h == 0:
                    if 1 + PF <= k + PF < K:
                        issue_in(k + PF)
                    sub_sq(k)
                mm(j)
            if 0 <= j - SQRT_LAG < J:
                sqrt_p(j - SQRT_LAG)
            if 0 <= j - ADD_LAG < J:
                add_out(j - ADD_LAG)

    nc.compile()
    return nc


def kernel(x: np.ndarray, **_unused) -> np.ndarray:
    x = np.asarray(x)
    assert x.shape == (B, S, N, C), x.shape

    if "nc" not in _cache:
        _cache["nc"] = _build()
    nc = _cache["nc"]

    # Host-side repack: fp16, per (batch, half) partition a c-planar
    # [3, SH+1, 26] block; frame SH is the next real frame (half 0) or a
    # copy of the last frame (half 1) so the device-side distance at the
    # true sequence end is exactly 0.
    xh = np.ascontiguousarray(x).astype(np.float16)          # [B,S,25,3]
    ext = np.concatenate([xh, xh[:, -1:]], axis=1)           # [B,S+1,25,3]
    h0 = ext[:, 0:SH + 1]                                    # [B,513,25,3]
    h1 = ext[:, SH:S + 1]                                    # [B,513,25,3]
    hv = np.stack([h0, h1], axis=1)                          # [B,2,513,25,3]
    pl = np.transpose(hv, (0, 1, 4, 2, 3))                   # [B,2,3,513,25]
    buf = np.zeros((B, H, C, SH + 1, NP), np.float16)
    buf[..., :N] = pl

    eye = np.eye(P, dtype=np.float16).reshape(P * P)
    in_maps = [
        {
            "xin": buf[ci * BC:(ci + 1) * BC].reshape(IN_FLAT),
            "ident": eye,
        }
        for ci in range(NCORES)
    ]

    res = run_bass_kernel_spmd(nc, in_maps, core_ids=list(range(NCORES)))
    _cache["last_results"] = res

    out = np.empty((B, S, N, C), dtype=np.float32)
    for ci in range(NCORES):
        y = np.asarray(res.results[ci]["yout"]).reshape(BC, H, C, SH, NP)
        y = y[..., :N]                                       # strip node pad
        y = np.transpose(y, (0, 1, 3, 4, 2))                 # [BC,2,SH,25,3]
        out[ci * BC:(ci + 1) * BC] = y.reshape(BC, S, N, C).astype(np.float32)
    return out


# revision 10
# speedup vs baseline: 1.0684x; 1.0684x over previous
"""Trainium2 Bass kernel for nn_PositionalEncoding_61151744360729.

out[b, s, n, :] = x[b, s, n, :] + ||x[b, s+1, n, :] - x[b, s, n, :]||_2
(with distance 0 at s = S-1).

Sharding: data-parallel on batch across 8 NeuronCores (64 batches/core).

Device layout: fp16 end-to-end, c-planar. Host repacks x to, per
(batch, seq-half) partition, [3 coord planes][SH+1 frames][26 nodes]
(nodes padded 25->26 so the one-frame shift is 52B = 4B-aligned and all
DVE tensor_tensor ops hit the 2x perf mode; fp16 I/O halves HBM traffic
vs fp32).

v4 structure (SDMA pool ~55us of transfer is the roofline):
- One batched 3D-AP input DMA per chunk, one output DMA per half-chunk
  piece (the per-DMA ~600-900ns SP sequencer cost made 76 small DMAs a
  near-critical 48us in the original version).
- Input DMAs staggered (chunk k+2 issued at chunk k's compute): the
  SDMA engines round-robin across queued DMAs, so queueing everything
  up front starves chunk 0's completion by ~8us.
- The engines execute their queues in order, so a back-end op whose
  producer hasn't finished stalls everything behind it. With back-end
  ops issued right after the next front (1-piece lag), the serial
  cycle add -> sub -> square -> sqrt (~11us/chunk) dominates: that was
  the old version's real limiter. Here sqrt lags its matmuls by 2
  pieces and add/output lag 5 pieces, so every op's inputs are long
  done when the engine reaches it and engines stream at busy-rate.
- PSUM pieces are [P, 1024] fp32 = 2 banks, 4 in flight, so the deep
  lag fits the 8 PSUM banks.
- Per-plane DVE subtracts (a fused 3-run-AP subtract measured ~10%
  slower than 3 single-run ops), squares on ACT (fused 3-plane op)
  except a DVE plane-2 square on some chunks to balance engines, plane
  sum as identity matmuls into PSUM, ACT sqrt per piece, one stride-0
  broadcast DVE add per piece.
- No GPSIMD tensor ops: the Pool engine shares its SBUF port with the
  DVE, and measured contention inflated concurrent DVE ops 25-100%.
"""

import sys
from contextlib import ExitStack

for _p in ("/opt/trn_rl_repo", "/root/.axon_site/_ro/trn_rl_repo"):
    if _p not in sys.path:
        sys.path.insert(0, _p)

import numpy as np

import concourse.bass as bass
import concourse.tile as tile
from concourse import bacc, mybir
from concourse.bass_utils import run_bass_kernel_spmd

B, S, N, C = 512, 1024, 25, 3
NCORES = 8
BC = B // NCORES           # 64 batches per core
H = 2                      # sequence halves -> 128 partitions
SH = S // H                # 512 frames per half
P = H * BC                 # 128 partitions
NP = 26                    # nodes padded to 26 (4B-aligned frame stride)
IN_PLANE = (SH + 1) * NP   # input elems per coord plane per partition
OUT_PLANE = SH * NP        # output elems per plane per partition
IN_FLAT = P * C * IN_PLANE
OUT_FLAT = P * C * OUT_PLANE
PSUM_W = 512               # one PSUM bank of fp32 per matmul window

F = 64                     # frames per chunk
K = SH // F                # 8 chunks
FI = (F + 1) * NP          # 1690 input elems per plane per chunk
FD = F * NP                # 1664 output elems per plane per chunk
HW = FD // 2               # 832: half-chunk elems per plane

# square of plane 2 per chunk: 'act' (fused into the 3-plane ACT op) or
# 'dve' (separate DVE multiply) -- tuned so DVE and ACT busy time match
# (measured: fused 3-plane square costs ACT only ~1.1us over the 2-plane
# op while a DVE multiply costs ~1.0us, and DVE is the fuller engine)
SQ2 = ["act"] * 8

J = 2 * K                  # 16 half-chunk pieces
ADD_LAG = 5                # add+output of piece j at iteration j+ADD_LAG

_cache = {}


def _build():
    f16 = mybir.dt.float16
    f32 = mybir.dt.float32
    Af = mybir.ActivationFunctionType
    nc = bacc.Bacc(
        "TRN2", target_bir_lowering=False, debug=False, num_devices=NCORES
    )
    xin = nc.dram_tensor("xin", [IN_FLAT], f16, kind="ExternalInput")
    ident = nc.dram_tensor("ident", [P * P], f16, kind="ExternalInput")
    yout = nc.dram_tensor("yout", [OUT_FLAT], f16, kind="ExternalOutput")

    OFF = [k * F for k in range(K)]
    PF = 2                 # input prefetch depth in chunks

    with tile.TileContext(nc) as tc, ExitStack() as ctx:
        pconst = ctx.enter_context(tc.tile_pool(name="pconst", bufs=2))
        pin0 = ctx.enter_context(tc.tile_pool(name="pin0", bufs=3))
        pin = ctx.enter_context(tc.tile_pool(name="pin", bufs=6))
        pd = ctx.enter_context(tc.tile_pool(name="pd", bufs=2))
        ps = ctx.enter_context(tc.tile_pool(name="ps", bufs=4))
        po = ctx.enter_context(tc.tile_pool(name="po", bufs=4))
        ppsum = ctx.enter_context(
            tc.tile_pool(name="ppsum", bufs=2, space="PSUM")
        )

        # chunk 0 inputs first and per plane: they have the SDMA pool to
        # themselves while the SP sequencer works through later issues,
        # so the first subtract can start at ~3.5us
        x0 = []
        for c in range(C):
            t = pin0.tile([P, FI], f16)
            nc.sync.dma_start(
                t[:],
                bass.AP(xin, c * IN_PLANE, [[C * IN_PLANE, P], [1, FI]]),
            )
            x0.append(t)
        id_t = pconst.tile([P, P], f16)
        nc.sync.dma_start(id_t[:], bass.AP(ident, 0, [[P, P], [1, P]]))

        # dummy activations so both ACT function tables (square and
        # sqrt) load during the DMA fill instead of mid-stream (each
        # ACT_TABLE_LOAD is ~1.4us on the in-order ACT queue)
        scratch = pconst.tile([P, 2], f16)
        nc.scalar.activation(scratch[:], x0[0][:, 0:2], Af.Sqrt)
        nc.scalar.activation(scratch[:], x0[0][:, 0:2], Af.Square)

        xk = [None] * K
        dk = [None] * K
        psum_k = [None] * K
        dist_k = [None] * K

        def issue_in(k):
            t = pin.tile([P, C * FI], f16)
            src = bass.AP(
                xin,
                OFF[k] * NP,
                [[C * IN_PLANE, P], [IN_PLANE, C], [1, FI]],
            )
            nc.sync.dma_start(t[:], src)
            xk[k] = t

        for k in range(1, 1 + PF):
            issue_in(k)

        def sub_sq(k):
            """per-plane subs + squares for chunk k (both pieces)"""
            d_t = pd.tile([P, C * FD], f16)
            dk[k] = d_t
            d3 = d_t[:].rearrange("p (c x) -> p c x", c=C)
            if k == 0:
                for c in range(C):
                    nc.vector.tensor_sub(
                        d3[:, c], x0[c][:, NP:NP + FD], x0[c][:, 0:FD]
                    )
            else:
                x3 = xk[k][:].rearrange("p (c f) -> p c f", c=C)
                for c in range(C):
                    nc.vector.tensor_sub(
                        d3[:, c], x3[:, c, NP:], x3[:, c, 0:FD]
                    )
            if SQ2[k] == "act":
                nc.scalar.activation(
                    d_t[:, 0:C * FD], d_t[:, 0:C * FD], Af.Square
                )
            else:
                nc.scalar.activation(
                    d_t[:, 0:2 * FD], d_t[:, 0:2 * FD], Af.Square
                )
                sq2 = d3[:, 2]
                nc.vector.tensor_mul(sq2, sq2, sq2)

        def mm(k):
            """plane-sum matmuls for chunk k -> [P, 2048] 4-bank psum"""
            d3 = dk[k][:].rearrange("p (c x) -> p c x", c=C)
            ps_t = ppsum.tile([P, 2048], f32)
            psum_k[k] = ps_t
            for w0 in range(0, FD, PSUM_W):
                w1 = min(w0 + PSUM_W, FD)
                for c in range(C):
                    nc.tensor.matmul(
                        ps_t[:, w0:w1],
                        id_t[:],
                        d3[:, c, w0:w1],
                        start=(c == 0),
                        stop=(c == C - 1),
                    )

        def sqrt_k(k):
            s_t = ps.tile([P, FD], f16)
            dist_k[k] = s_t
            nc.scalar.activation(s_t[:], psum_k[k][:, 0:FD], Af.Sqrt)
            psum_k[k] = None

        def add_out(j):
            k, h = j // 2, j % 2
            lo, hi = h * HW, (h + 1) * HW
            s_t = dist_k[k]
            o_t = po.tile([P, C * HW], f16)
            o3 = o_t[:].rearrange("p (c x) -> p c x", c=C)
            if k == 0:
                for c in range(C):
                    nc.vector.tensor_add(
                        o3[:, c], x0[c][:, lo:hi], s_t[:, lo:hi]
                    )
            else:
                x3 = xk[k][:].rearrange("p (c f) -> p c f", c=C)
                sb = s_t[:, lo:hi].unsqueeze(1).broadcast_to([P, C, HW])
                nc.vector.tensor_add(o3, x3[:, :, lo:hi], sb)
            dst = bass.AP(
                yout,
                OFF[k] * NP + lo,
                [[C * OUT_PLANE, P], [OUT_PLANE, C], [1, HW]],
            )
            nc.sync.dma_start(dst, o3)

        # deep-lag software pipeline: fronts at chunk granularity on the
        # even iterations, sqrt one piece behind its matmuls, add+output
        # per piece ADD_LAG pieces behind
        for j in range(J + ADD_LAG + 1):
            if j % 2 == 0 and j // 2 < K:
                k = j // 2
                if 1 + PF <= k + PF < K:
                    issue_in(k + PF)
                sub_sq(k)
                mm(k)
            elif j % 2 == 1 and (j - 1) // 2 < K:
                sqrt_k((j - 1) // 2)
            if 0 <= j - ADD_LAG < J:
                add_out(j - ADD_LAG)

    nc.compile()
    return nc


def kernel(x: np.ndarray, **_unused) -> np.ndarray:
    x = np.asarray(x)
    assert x.shape == (B, S, N, C), x.shape

    if "nc" not in _cache:
        _cache["nc"] = _build()
    nc = _cache["nc"]

    # Host-side repack: fp16, per (batch, half) partition a c-planar
    # [3, SH+1, 26] block; frame SH is the next real frame (half 0) or a
    # copy of the last frame (half 1) so the device-side distance at the
    # true sequence end is exactly 0.
    xh = np.ascontiguousarray(x).astype(np.float16)          # [B,S,25,3]
    ext = np.concatenate([xh, xh[:, -1:]], axis=1)           # [B,S+1,25,3]
    h0 = ext[:, 0:SH + 1]                                    # [B,513,25,3]
    h1 = ext[:, SH:S + 1]                                    # [B,513,25,3]
    hv = np.stack([h0, h1], axis=1)                          # [B,2,513,25,3]
    pl = np.transpose(hv, (0, 1, 4, 2, 3))                   # [B,2,3,513,25]
    buf = np.zeros((B, H, C, SH + 1, NP), np.float16)
    buf[..., :N] = pl

    eye = np.eye(P, dtype=np.float16).reshape(P * P)
    in_maps = [
        {
            "xin": buf[ci * BC:(ci + 1) * BC].reshape(IN_FLAT),
            "ident": eye,
        }
        for ci in range(NCORES)
    ]

    res = run_bass_kernel_spmd(nc, in_maps, core_ids=list(range(NCORES)))
    _cache["last_results"] = res

    out = np.empty((B, S, N, C), dtype=np.float32)
    for ci in range(NCORES):
        y = np.asarray(res.results[ci]["yout"]).reshape(BC, H, C, SH, NP)
        y = y[..., :N]                                       # strip node pad
        y = np.transpose(y, (0, 1, 3, 4, 2))                 # [BC,2,SH,25,3]
        out[ci * BC:(ci + 1) * BC] = y.reshape(BC, S, N, C).astype(np.float32)
    return out
